# revision 1
# baseline (speedup 1.0000x reference)
"""Trainium2 Bass kernel for nn_KanBoard768 (KAN network forward pass).

Data-parallel across 8 NeuronCores: batch 32768 -> 4096 rows/core, weights
replicated, no collectives.

Math: the cubic B-spline bases are reformulated as truncated powers,
    N(u - j) = (1/6) * sum_r (-1)^r C(4,r) relu(u - j - r)^3
so the spline matmul becomes  sum_{e,s} D[o,e,s] * relu(u_e - s)^3  with the
binomial transform folded into D on the host.  The relu-cube features are
produced by a fused custom DVE op (mul, sub, relu, sq, mul = 5 ALU stages)
reading the hidden activations straight from PSUM, with the grid transform
u = (x + ft_b - g0)/h folded into the op's scale and per-partition shift.
"""

import numpy as np

# --- problem constants (hardcoded; kernel.py must be self-contained) ---
GRID_SIZE, SPLINE_ORDER = 5, 3
H = 2.0 / GRID_SIZE                    # 0.4
G0 = -SPLINE_ORDER * H - 1.0           # -2.2
INV_H = 1.0 / H                        # 2.5 (exact in fp32)
NB = GRID_SIZE + SPLINE_ORDER          # 8 bases per edge
NS = GRID_SIZE + 2 * SPLINE_ORDER + 1  # 12 truncated-power shifts
B, IN_FT, HID = 32768, 768, 128
NCORES = 8
BC = B // NCORES                       # 4096 rows per core
NT = 512                               # batch tile (one PSUM bank of fp32)
NBT = BC // NT                         # 8 batch tiles per core
KT_FT = IN_FT // 128                   # 6 contraction tiles for the ft layer

_CACHE = {}


def _register_relu_cube():
    import concourse.dve_ops as dve_ops
    from concourse.dve_spec import Spec, Src0, C0, C2, relu, sq, lower
    from concourse.dve_uop import DveOpSpec

    name = "RELU_CUBE_AFF_ANT"
    for op in dve_ops.OPS:
        if op.name == name:
            return op
    r = relu(Src0 * C2 - C0)
    spec = Spec(
        body=sq(r) * r,
        reference=lambda in0, in1, s0, s1, imm2: np.maximum(
            in0.astype(np.float32) * imm2 - s0, 0.0
        )
        ** 3,
    )
    row = dve_ops._CUSTOM_DVE_ROW_BASE + len(dve_ops.OPS)
    assert row < 0x20
    shas = {}
    for ver in ("v3", "v4"):
        try:
            shas[ver] = DveOpSpec(
                name=name, opcode=row, uops=lower(spec, ver=ver), rd1_en=False
            ).sha(ver)
        except Exception:
            pass
    op = dve_ops.DveOp(name, spec, subdim=False, uops_sha=shas)
    dve_ops.OPS.append(op)
    dve_ops._SUB_OPCODE_FOR_NAME[name] = row
    dve_ops.CUSTOM_DVE_SPECS[name] = spec
    return op


def _build_module():
    if "nc" in _CACHE:
        return _CACHE["nc"]
    from contextlib import ExitStack

    import concourse.bass as bass
    import concourse.mybir as mybir
    import concourse.tile as tile
    from concourse import bacc

    RELU_CUBE = _register_relu_cube()
    AF = mybir.ActivationFunctionType
    f32 = mybir.dt.float32

    nc = bacc.Bacc("TRN2", target_bir_lowering=False, debug=False)

    stmT = nc.dram_tensor("stm_t", (IN_FT, BC), f32, kind="ExternalInput").ap()
    nstmT = nc.dram_tensor("nstm_t", (IN_FT, BC), f32, kind="ExternalInput").ap()
    wft = nc.dram_tensor("wft", (KT_FT, 128, 128), f32, kind="ExternalInput").ap()
    d1 = nc.dram_tensor("d1", (2 * NS, 128, 128), f32, kind="ExternalInput").ap()
    b1 = nc.dram_tensor("b1", (2, 128, 128), f32, kind="ExternalInput").ap()
    d2 = nc.dram_tensor("d2", (NS + 1, 128, 1), f32, kind="ExternalInput").ap()
    sh1 = nc.dram_tensor("sh1", (128, NS), f32, kind="ExternalInput").ap()
    ftb = nc.dram_tensor("ftb", (128, 1), f32, kind="ExternalInput").ap()
    out_d = nc.dram_tensor("out", (1, BC), f32, kind="ExternalOutput").ap()

    with tile.TileContext(nc) as tc, ExitStack() as ctx:
        wpool = ctx.enter_context(tc.tile_pool(name="weights", bufs=1))
        inpool = ctx.enter_context(tc.tile_pool(name="inp", bufs=3))
        spool = ctx.enter_context(tc.tile_pool(name="silu", bufs=3))
        fpool = ctx.enter_context(tc.tile_pool(name="feats", bufs=32))
        opool = ctx.enter_context(tc.tile_pool(name="outb", bufs=1))
        pspool = ctx.enter_context(tc.tile_pool(name="ps", bufs=2, space="PSUM"))
        popool = ctx.enter_context(tc.tile_pool(name="pso", bufs=2, space="PSUM"))

        wft_sb = wpool.tile([128, KT_FT, 128], f32)
        nc.sync.dma_start(wft_sb[:], wft.rearrange("k p m -> p k m"))
        d1_sb = wpool.tile([128, 2 * NS, 128], f32)
        nc.sync.dma_start(d1_sb[:], d1.rearrange("k p m -> p k m"))
        b1_sb = wpool.tile([128, 2, 128], f32)
        nc.sync.dma_start(b1_sb[:], b1.rearrange("k p m -> p k m"))
        d2_sb = wpool.tile([128, NS + 1, 1], f32)
        nc.sync.dma_start(d2_sb[:], d2.rearrange("k p m -> p k m"))
        sh1_sb = wpool.tile([128, NS], f32)
        nc.sync.dma_start(sh1_sb[:], sh1[:])
        ftb_sb = wpool.tile([128, 1], f32)
        nc.sync.dma_start(ftb_sb[:], ftb[:])

        outbuf = opool.tile([1, BC], f32)
        out_sig = opool.tile([1, BC], f32)

        stmT_r = stmT.rearrange("(k p) n -> p k n", p=128)
        nstmT_r = nstmT.rearrange("(k p) n -> p k n", p=128)

        for bt in range(NBT):
            sl = bass.ts(bt, NT)
            xs = inpool.tile([128, KT_FT, NT], f32, tag="xs")
            nc.sync.dma_start(xs[:], stmT_r[:, :, sl])
            xn = inpool.tile([128, KT_FT, NT], f32, tag="xn")
            nc.sync.dma_start(xn[:], nstmT_r[:, :, sl])

            ps_s = pspool.tile([128, NT], f32, tag="ps_s")
            ps_n = pspool.tile([128, NT], f32, tag="ps_n")
            for k in range(KT_FT):
                nc.tensor.matmul(
                    ps_s[:], wft_sb[:, k, :], xs[:, k, :],
                    start=(k == 0), stop=(k == KT_FT - 1),
                )
            for k in range(KT_FT):
                nc.tensor.matmul(
                    ps_n[:], wft_sb[:, k, :], xn[:, k, :],
                    start=(k == 0), stop=(k == KT_FT - 1),
                )

            silu_s = spool.tile([128, NT], f32, tag="sl_s")
            nc.scalar.activation(silu_s[:], ps_s[:], AF.Silu, bias=ftb_sb[:])
            silu_n = spool.tile([128, NT], f32, tag="sl_n")
            nc.scalar.activation(silu_n[:], ps_n[:], AF.Silu, bias=ftb_sb[:])

            ps_h2 = pspool.tile([128, NT], f32, tag="ps_h2")
            mmi = 0
            for half, ps_x in ((0, ps_s), (1, ps_n)):
                for s in range(NS):
                    f = fpool.tile([128, NT], f32, tag="feat")
                    nc.vector._custom_dve(
                        RELU_CUBE, out=f[:], in0=ps_x[:],
                        s0=sh1_sb[:, s : s + 1], imm2=INV_H,
                    )
                    nc.tensor.matmul(
                        ps_h2[:], d1_sb[:, half * NS + s, :], f[:],
                        start=(mmi == 0), stop=False,
                    )
                    mmi += 1
            nc.tensor.matmul(ps_h2[:], b1_sb[:, 0, :], silu_s[:], start=False, stop=False)
            nc.tensor.matmul(ps_h2[:], b1_sb[:, 1, :], silu_n[:], start=False, stop=True)

            silu2 = spool.tile([128, NT], f32, tag="sl2")
            nc.scalar.activation(silu2[:], ps_h2[:], AF.Silu, bias=0.0)

            ps_o = popool.tile([1, NT], f32, tag="ps_o")
            for s in range(NS):
                f2 = fpool.tile([128, NT], f32, tag="feat")
                nc.vector._custom_dve(
                    RELU_CUBE, out=f2[:], in0=ps_h2[:],
                    s0=float(s + G0 * INV_H), imm2=INV_H,
                )
                nc.tensor.matmul(
                    ps_o[:], d2_sb[:, s, :], f2[:], start=(s == 0), stop=False
                )
            nc.tensor.matmul(ps_o[:], d2_sb[:, NS, :], silu2[:], start=False, stop=True)

            nc.vector.tensor_copy(outbuf[:, sl], ps_o[:])

        nc.scalar.activation(out_sig[:], outbuf[:], AF.Sigmoid, bias=0.0)
        nc.sync.dma_start(out_d[:], out_sig[:])

    nc.compile()
    _CACHE["nc"] = nc
    return nc


def _make_D(spline_w):
    # spline_w: (out, in, NB) -> D: (out, in, NS) via the binomial transform
    out, inn, nb = spline_w.shape
    C4 = np.array([1.0, -4.0, 6.0, -4.0, 1.0], dtype=np.float64) / 6.0
    D = np.zeros((out, inn, NS), dtype=np.float64)
    sw = spline_w.astype(np.float64)
    for j in range(NB):
        for r in range(5):
            D[:, :, j + r] += C4[r] * sw[:, :, j]
    return D.astype(np.float32)


def _host_prep(inputs):
    stm = np.asarray(inputs["stm"], dtype=np.float32)
    nstm = np.asarray(inputs["nstm"], dtype=np.float32)
    ft_w = np.asarray(inputs["ft_w"], dtype=np.float32)
    ft_b = np.asarray(inputs["ft_b"], dtype=np.float32)
    w1b = np.asarray(inputs["kan1_base_w"], dtype=np.float32)
    w1s = np.asarray(inputs["kan1_spline_w"], dtype=np.float32)
    w2b = np.asarray(inputs["kan2_base_w"], dtype=np.float32)
    w2s = np.asarray(inputs["kan2_spline_w"], dtype=np.float32)

    stmT = np.ascontiguousarray(stm.T)    # (768, B)
    nstmT = np.ascontiguousarray(nstm.T)

    # ft layer: lhsT[k, m] = ft_w[m, k] -> tiles (KT, 128, 128)
    wft_np = np.ascontiguousarray(ft_w.T.reshape(KT_FT, 128, HID))

    # kan1 spline: D1 (128, 256, NS); lhsT tile [e, o] per (half, s)
    D1 = _make_D(w1s)
    d1_np = np.empty((2 * NS, 128, 128), dtype=np.float32)
    for half in range(2):
        for s in range(NS):
            d1_np[half * NS + s] = D1[:, half * 128 : (half + 1) * 128, s].T
    b1_np = np.stack([w1b[:, :128].T, w1b[:, 128:].T]).astype(np.float32)

    # kan2: D2 (1, 128, NS) -> columns [e2, 1]; last slot = base weights
    D2 = _make_D(w2s)
    d2_np = np.empty((NS + 1, 128, 1), dtype=np.float32)
    for s in range(NS):
        d2_np[s, :, 0] = D2[0, :, s]
    d2_np[NS, :, 0] = w2b[0, :]

    # per-partition shift vector for layer-1 features: u = x*INV_H + bv,
    # t = u - s = x*INV_H - (s - bv);  bv = (ft_b - G0)/H
    bv = (ft_b.astype(np.float64) - G0) / H
    sh1_np = (
        np.arange(NS, dtype=np.float64)[None, :] - bv[:, None]
    ).astype(np.float32)
    ftb_np = ft_b.reshape(128, 1).astype(np.float32)

    weights = dict(
        wft=wft_np, d1=d1_np, b1=b1_np, d2=d2_np, sh1=sh1_np, ftb=ftb_np
    )
    return stmT, nstmT, weights


def kernel(**inputs):
    from concourse.bass_utils import run_bass_kernel_spmd

    nc = _build_module()
    stmT, nstmT, weights = _host_prep(inputs)

    in_maps = []
    for c in range(NCORES):
        sl = slice(c * BC, (c + 1) * BC)
        m = {
            "stm_t": np.ascontiguousarray(stmT[:, sl]),
            "nstm_t": np.ascontiguousarray(nstmT[:, sl]),
        }
        m.update(weights)
        in_maps.append(m)

    res = run_bass_kernel_spmd(nc, in_maps, core_ids=list(range(NCORES)))
    out = np.concatenate([r["out"].reshape(-1) for r in res.results])
    return out.reshape(B, 1).astype(np.float32)


if __name__ == "__main__":
    rng = np.random.default_rng(0)
    nb = NB
    fake = {
        "stm": rng.random((B, IN_FT), dtype=np.float32),
        "nstm": rng.random((B, IN_FT), dtype=np.float32),
        "ft_w": (rng.standard_normal((HID, IN_FT)) * 0.02).astype(np.float32),
        "ft_b": np.zeros(HID, np.float32),
        "kan1_base_w": (rng.standard_normal((HID, 2 * HID)) * 0.05).astype(np.float32),
        "kan1_spline_w": (rng.standard_normal((HID, 2 * HID, nb)) * 0.05).astype(np.float32),
        "kan2_base_w": (rng.standard_normal((1, HID)) * 0.05).astype(np.float32),
        "kan2_spline_w": (rng.standard_normal((1, HID, nb)) * 0.05).astype(np.float32),
    }
    out = kernel(**fake)
    print("kernel out", out.shape, out.dtype, out[:5, 0])
